# revision 6
# baseline (speedup 1.0000x reference)
"""Dark-Channel-Prior dehazing (DCPGenerator) Trainium2 Bass kernel.

Contract: kernel(x: [16,3,512,512] f32) -> [16,3,512,512] f32.
Data-parallel over 8 NeuronCores: 2 samples per core. Each core runs the
full per-sample pipeline on-device:
  guidance/img prep -> dark channel (15x15 min-pool, bf16) -> atmospheric
  light (top-1% selection via secant-estimated threshold + band-corrected
  mean, bf16 counting) -> second dark channel on img/A (bf16) -> guided
  filter (r=40 box sums via free-dim scans + fp32r banded-matmul partition
  sums) -> output.
"""
import numpy as np
from contextlib import ExitStack

H = 512
W = 512
NCHUNK = 4          # 4 row-chunks of 128 partitions
CW = 512            # chunk free width
PADW = 526          # padded chunk width for the 15-wide min pool (7+512+7)
CUMW = 593          # hbox cum chunk: 41 zeros | 512 cumsum | 40 x cum[511]
WIN_PAD = 7
RADIUS = 40
EPS = 1e-3
OMEGA = 0.95
TOPN = int(0.01 * H * W)          # 2621
T0 = 0.0055                       # secant bracket on raw-x dark scale
T1 = 0.0085
BAND = 2e-4                       # band width for tie-region correction
SECANT_ROUNDS = 6

_CACHE = {}


# ---------------------------------------------------------------- host consts
def _host_consts():
    n1 = np.minimum(np.arange(H) + RADIUS, H - 1) - np.maximum(np.arange(H) - RADIUS, 0) + 1
    inv_nh = (1.0 / n1).astype(np.float32)          # [512]
    inv_nw = inv_nh.copy()                          # same for W=512
    invnh = np.zeros((128, NCHUNK), np.float32)
    for c in range(NCHUNK):
        invnh[:, c] = inv_nh[c * 128:(c + 1) * 128]
    invnw_rep = np.broadcast_to(inv_nw[None, :], (128, W)).copy()
    k = np.arange(128)[:, None]
    p = np.arange(128)[None, :]
    band = (np.abs(k - p) <= RADIUS).astype(np.float32)
    bu = (k >= p + 128 - RADIUS).astype(np.float32) / 81.0
    bd = (k <= p - (128 - RADIUS)).astype(np.float32) / 81.0
    bms = []
    for c in range(NCHUNK):
        bms.append(band * inv_nh[c * 128:(c + 1) * 128][None, :])
    return {"invnh": invnh, "invnw": invnw_rep,
            "bm0": bms[0], "bm1": bms[1], "bm3": bms[3], "bu": bu, "bd": bd}


# ------------------------------------------------------------------ program
def _build():
    import concourse.bacc as bacc
    import concourse.tile as tile
    import concourse.bass as bass
    from concourse import mybir

    f32 = mybir.dt.float32
    f32r = mybir.dt.float32r
    bf16 = mybir.dt.bfloat16
    Alu = mybir.AluOpType
    Act = mybir.ActivationFunctionType

    nc = bacc.Bacc("TRN2", target_bir_lowering=False, debug=False, num_devices=8)

    x_ext = nc.dram_tensor("x", [2, 3, H, W], f32, kind="ExternalInput").ap()
    band_exts = {nm: nc.dram_tensor(nm, [128, 128], f32, kind="ExternalInput").ap()
                 for nm in ("bm0", "bm1", "bm3", "bu", "bd")}
    invnh_ext = nc.dram_tensor("invnh", [128, NCHUNK], f32, kind="ExternalInput").ap()
    invnw_ext = nc.dram_tensor("invnw", [128, W], f32, kind="ExternalInput").ap()
    y_ext = nc.dram_tensor("y", [2, 3, H, W], f32, kind="ExternalOutput").ap()

    def cview(t, width=CW):
        """[128, NCHUNK*width] tile -> [128, NCHUNK, width] view."""
        return t.rearrange("p (c w) -> p c w", w=width)

    def fbcast(ap_col, n):
        """free-dim step-0 broadcast of a [...,1] AP to [...,n]."""
        return bass.AP(tensor=ap_col.tensor, offset=ap_col.offset,
                       ap=[list(p) for p in ap_col.ap[:-1]] + [[0, n]])

    with ExitStack() as ctx:
        tc = ctx.enter_context(tile.TileContext(nc))

        cpool = ctx.enter_context(tc.tile_pool(name="cpool", bufs=1))
        big = ctx.enter_context(tc.tile_pool(name="big", bufs=1))
        pp = ctx.enter_context(tc.tile_pool(name="pp", bufs=1))       # minpool / box scratch
        boxes = ctx.enter_context(tc.tile_pool(name="boxes", bufs=5))
        srcp = ctx.enter_context(tc.tile_pool(name="srcp", bufs=3))
        abt = ctx.enter_context(tc.tile_pool(name="abt", bufs=3))
        tiny = ctx.enter_context(tc.tile_pool(name="tiny", bufs=2))
        pbig = ctx.enter_context(tc.tile_pool(name="pbig", bufs=2, space="PSUM"))
        psml = ctx.enter_context(tc.tile_pool(name="psml", bufs=2, space="PSUM"))

        # ---- constants ----
        c_band = {}
        stage = cpool.tile([128, 128], f32, name="s_band")
        for nm in ("bm0", "bm1", "bm3", "bu", "bd"):
            nc.sync.dma_start(out=stage[:], in_=band_exts[nm][:])
            c_band[nm] = cpool.tile([128, 128], f32r, name=f"c_{nm}")
            nc.scalar.copy(c_band[nm][:], stage[:])
        c_bm = [c_band["bm0"], c_band["bm1"], c_band["bm1"], c_band["bm3"]]
        c_invnh = cpool.tile([128, NCHUNK], f32, name="c_invnh")
        nc.sync.dma_start(out=c_invnh[:], in_=invnh_ext[:])
        c_invnw = cpool.tile([128, W], f32, name="c_invnw")
        nc.sync.dma_start(out=c_invnw[:], in_=invnw_ext[:])
        c_ones128 = cpool.tile([128, 1], f32, name="c_ones128")
        nc.vector.memset(c_ones128[:], 1.0)
        c_ones1x = cpool.tile([1, 128], f32, name="c_ones1x")
        nc.vector.memset(c_ones1x[:], 1.0)
        c_zeros = cpool.tile([128, CW], f32, name="c_zeros")
        nc.vector.memset(c_zeros[:], 0.0)
        c_ones16 = cpool.tile([128, CW], bf16, name="c_ones16")
        nc.vector.memset(c_ones16[:], 1.0)

        # ------------------------------------------------ helpers (emit ops)
        def interior(t):
            """padded tile -> [128, NCHUNK, CW] strided view of the interiors."""
            return cview(t, PADW)[:, :, WIN_PAD:WIN_PAD + CW]

        def memset_pads(t, eng):
            v = cview(t, PADW)
            eng.memset(v[:, :, 0:WIN_PAD], 1.0)
            eng.memset(v[:, :, PADW - WIN_PAD:PADW], 1.0)

        def hpool(dst, padded, w1, eng):
            """15-wide sliding min along free dim; padded [128,4*526] -> dst [128,4*512].
            Single multi-dim-AP instructions across all 4 chunks."""
            a = cview(padded, PADW)
            b = cview(w1, PADW)
            d = cview(dst)
            eng.tensor_tensor(b[:, :, 0:525], a[:, :, 0:525], a[:, :, 1:526], Alu.min)
            eng.tensor_tensor(a[:, :, 0:523], b[:, :, 0:523], b[:, :, 2:525], Alu.min)
            eng.tensor_tensor(b[:, :, 0:519], a[:, :, 0:519], a[:, :, 4:523], Alu.min)
            eng.tensor_tensor(d[:, :, :], b[:, :, 0:512], b[:, :, 7:519], Alu.min)

        def vshift_dma(dst, src, s, pad_tile):
            """dst[row r] = src[row r+s] (global 512-row space), bottom s rows from pad."""
            dv, sv = cview(dst), cview(src)
            nc.sync.dma_start(out=dv[0:128 - s, :, :], in_=sv[s:128, :, :])
            nc.sync.dma_start(out=dv[128 - s:128, 0:NCHUNK - 1, :],
                              in_=sv[0:s, 1:NCHUNK, :])
            nc.sync.dma_start(out=dv[128 - s:128, NCHUNK - 1, :], in_=pad_tile[0:s, :])

        def vshift_dma_down(dst, src, s, pad_tile):
            """dst[row r] = src[max(r-s, 0)] (clamped at the top edge)."""
            dv, sv = cview(dst), cview(src)
            nc.sync.dma_start(out=dv[s:128, :, :], in_=sv[0:128 - s, :, :])
            nc.sync.dma_start(out=dv[0:s, 1:NCHUNK, :],
                              in_=sv[128 - s:128, 0:NCHUNK - 1, :])
            for k in range(s):
                nc.sync.dma_start(out=dv[k:k + 1, 0, :], in_=sv[0:1, 0, :])

        def hbox(dst, src, cum, eng_scan, eng_elem):
            """zero-padded 81-wide box sum along free dim. src [128,2048];
            dst [128,2048] (f32r); cum [128, 4*593] with per-chunk layout
            [41 zeros|512 cum|40 rep] (zeros pre-set once per sample)."""
            sv, dv, cv = cview(src), cview(dst), cview(cum, CUMW)
            for c in range(NCHUNK):
                eng_scan.tensor_tensor_scan(cv[:, c, 41:553], sv[:, c, :], c_zeros[:],
                                            0.0, Alu.add, Alu.add)
            for c in range(NCHUNK):
                eng_elem.tensor_copy(cv[:, c, 553:593], fbcast(cv[:, c, 552:553], 40))
            eng_elem.tensor_tensor(dv[:, :, :], cv[:, :, 81:593], cv[:, :, 0:512],
                                   Alu.subtract)

        def vbox(dst, src, eng=None):
            """mean over the 81-tall zero-padded vertical box, with invNh and
            invNw folded in: dst = (banded_matmul(src)) * invNw. src is f32r.
            PSUM evacuation must be V or Scalar (gpsimd has no PSUM port)."""
            sv, dv = cview(src), cview(dst)
            for c in range(NCHUNK):
                ops = []
                if c > 0:
                    ops.append((c_band["bu"], c - 1))
                ops.append((c_bm[c], c))
                if c < NCHUNK - 1:
                    ops.append((c_band["bd"], c + 1))
                ps = pbig.tile([128, CW], f32, name="vps", tag="vps")
                for i, (mat, sc_) in enumerate(ops):
                    nc.tensor.matmul(ps[:], mat[:], sv[:, sc_, :],
                                     start=(i == 0), stop=(i == len(ops) - 1))
                nc.vector.tensor_tensor(dv[:, c, :], ps[:], c_invnw[:], Alu.mult)

        # ======================================================== per sample
        for s in range(2):
            V = nc.vector
            G = nc.gpsimd

            # ---- load ----
            xch = []
            for chn in range(3):
                t = big.tile([128, NCHUNK * CW], f32, name=f"x{chn}", tag=f"x{chn}")
                for c in range(NCHUNK):
                    nc.sync.dma_start(out=cview(t)[:, c, :],
                                      in_=x_ext[s, chn, c * 128:(c + 1) * 128, :])
                xch.append(t)
            xr, xg, xb = xch

            # ---- bf16 channel copies (for dark1 + masked sums) ----
            x16 = []
            for chn, xt in enumerate(xch):
                t16 = srcp.tile([128, NCHUNK * CW], bf16, name=f"x16_{chn}",
                                tag="srcp")
                nc.scalar.activation(t16[:], xt[:], Act.Copy, bias=0.0, scale=1.0)
                x16.append(t16)
            xr16, xg16, xb16 = x16

            # ---- guidance I = ((.2989 xr + .587 xg + .114 xb) + 1)/2 (f32) ----
            Ia = pp.tile([128, NCHUNK * CW], f32, name="Ia", tag="shv")
            Ib = pp.tile([128, NCHUNK * CW], f32, name="Ib", tag="Ib")
            nc.scalar.activation(Ib[:], xr[:], Act.Copy, bias=0.5, scale=0.14945)
            V.scalar_tensor_tensor(Ia[:], xg[:], 0.2935, Ib[:], Alu.mult, Alu.add)
            V.scalar_tensor_tensor(Ib[:], xb[:], 0.057, Ia[:], Alu.mult, Alu.add)
            guid = Ib  # final guidance lives in Ib; Ia free for reuse

            # ---- dark1 = minpool15(min_c x) in bf16 ----
            mxp = pp.tile([128, NCHUNK * PADW], bf16, name="mxp", tag="mxp")
            w1 = pp.tile([128, NCHUNK * PADW], bf16, name="w1", tag="w1")
            memset_pads(mxp, G)
            V.tensor_tensor(interior(mxp), cview(xr16)[:, :, :],
                            cview(xg16)[:, :, :], Alu.min)
            V.tensor_tensor(interior(mxp), interior(mxp),
                            cview(xb16)[:, :, :], Alu.min)
            uh = pp.tile([128, NCHUNK * CW], bf16, name="uh", tag="uh")
            hpool(uh, mxp, w1, V)
            sh = pp.tile([128, NCHUNK * CW], bf16, name="sh", tag="shv")
            u2 = pp.tile([128, NCHUNK * CW], bf16, name="u2", tag="u2")

            def cmin(dst_t, a_t2, b_t2, eng=V):
                eng.tensor_tensor(dst_t[:], a_t2[:], b_t2[:], Alu.min)

            vshift_dma(sh, uh, 1, c_ones16)
            cmin(u2, uh, sh)
            vshift_dma(sh, u2, 2, c_ones16)
            cmin(uh, u2, sh)
            vshift_dma(sh, uh, 4, c_ones16)
            cmin(u2, uh, sh)
            vshift_dma_down(sh, u2, 7, c_ones16)
            u = uh
            cmin(u, u2, sh)

            # ---- atmospheric light (bf16 counting / masked sums) ----
            junk = pp.tile([128, NCHUNK * CW], bf16, name="junk", tag="mxp")
            acc8 = tiny.tile([128, 8], f32, name="acc8", tag="acc8")
            V.memset(acc8[:], 0.0)
            thr = tiny.tile([128, 1], f32, name="thr", tag="thr")
            scal = tiny.tile([1, 16], f32, name="scal", tag="scal")
            V.memset(scal[:], 0.0)
            # scal cols: 0 ta, 1 Ca, 2 tb, 3 Cb, 4..temp
            V.memset(scal[:, 0:1], T0)
            V.memset(scal[:, 2:3], T1)

            def count_into(col):
                V.tensor_scalar(junk[:], u[:], thr[:], 0.0, Alu.is_gt, Alu.add,
                                accum_out=acc8[:, col:col + 1])
                fps = psml.tile([1, 1], f32, name="fold_ps", tag="fold_ps")
                nc.tensor.matmul(fps[:], c_ones128[:], acc8[:, col:col + 1],
                                 start=True, stop=True)
                return fps

            def bcast_thr(src_col):
                bp = psml.tile([128, 1], f32, name="thr_ps", tag="fold_ps")
                nc.tensor.matmul(bp[:], c_ones1x[:], src_col, start=True, stop=True)
                nc.scalar.copy(thr[:], bp[:])

            # C(t0), C(t1)
            bcast_thr(scal[0:1, 0:1])
            f = count_into(0)
            nc.scalar.copy(scal[:, 1:2], f[:])
            bcast_thr(scal[0:1, 2:3])
            f = count_into(0)
            nc.scalar.copy(scal[:, 3:4], f[:])
            for _rnd in range(SECANT_ROUNDS):
                # count is monotone non-increasing in t, so sign(dC) = -sign(dT);
                # step = (R - Cb) * dT/dC = (Cb - R) * |dT| / max(|dC|, 1)
                V.tensor_tensor(scal[:, 4:5], scal[:, 3:4], scal[:, 1:2], Alu.subtract)
                V.tensor_scalar(scal[:, 8:9], scal[:, 4:5], -1.0, 0.0, Alu.mult, Alu.add)
                V.tensor_tensor(scal[:, 4:5], scal[:, 4:5], scal[:, 8:9], Alu.max)
                V.tensor_scalar(scal[:, 4:5], scal[:, 4:5], 1.0, 0.0, Alu.max, Alu.add)
                V.tensor_tensor(scal[:, 5:6], scal[:, 2:3], scal[:, 0:1], Alu.subtract)
                V.tensor_scalar(scal[:, 8:9], scal[:, 5:6], -1.0, 0.0, Alu.mult, Alu.add)
                V.tensor_tensor(scal[:, 5:6], scal[:, 5:6], scal[:, 8:9], Alu.max)
                V.reciprocal(scal[:, 8:9], scal[:, 4:5])
                V.tensor_tensor(scal[:, 5:6], scal[:, 5:6], scal[:, 8:9], Alu.mult)
                V.tensor_scalar(scal[:, 6:7], scal[:, 3:4], 1.0, -float(TOPN),
                                Alu.mult, Alu.add)
                V.tensor_tensor(scal[:, 6:7], scal[:, 6:7], scal[:, 5:6], Alu.mult)
                V.tensor_copy(scal[:, 0:1], scal[:, 2:3])
                V.tensor_copy(scal[:, 1:2], scal[:, 3:4])
                V.tensor_tensor(scal[:, 2:3], scal[:, 2:3], scal[:, 6:7], Alu.add)
                bcast_thr(scal[0:1, 2:3])
                f = count_into(0)
                nc.scalar.copy(scal[:, 3:4], f[:])
            # final: C* already in acc8 col0 / scal3. masked sums at thr.
            for chn, xt in enumerate(x16):
                V.scalar_tensor_tensor(junk[:], u[:], thr[:], xt[:], Alu.is_gt,
                                       Alu.mult, accum_out=acc8[:, 1 + chn:2 + chn])
            # band threshold = thr - BAND
            V.tensor_scalar(scal[:, 7:8], scal[:, 2:3], 1.0, -BAND, Alu.mult, Alu.add)
            bcast_thr(scal[0:1, 7:8])
            V.tensor_scalar(junk[:], u[:], thr[:], 0.0, Alu.is_gt, Alu.add,
                            accum_out=acc8[:, 4:5])
            for chn, xt in enumerate(x16):
                V.scalar_tensor_tensor(junk[:], u[:], thr[:], xt[:], Alu.is_gt,
                                       Alu.mult, accum_out=acc8[:, 5 + chn:6 + chn])
            tps = psml.tile([1, 8], f32, name="tot_ps", tag="fold_ps")
            nc.tensor.matmul(tps[:], c_ones128[:], acc8[:], start=True, stop=True)
            tot = tiny.tile([1, 8], f32, name="tot", tag="tot")
            nc.scalar.copy(tot[:], tps[:])
            # A math: tot = [C*, Sr, Sg, Sb, Cb, Sbr, Sbg, Sbb]
            am = tiny.tile([1, 12], f32, name="am", tag="am")
            # am cols: 0:3 A_img, 3:6 recipA2, 6:9 bias_d (0.5-A), 9 amt, 10 recdc, 11 tmp
            V.tensor_tensor(am[:, 0:3], tot[:, 5:8], tot[:, 1:4], Alu.subtract)  # dS
            V.tensor_tensor(am[:, 11:12], tot[:, 4:5], tot[:, 0:1], Alu.subtract)  # dC
            V.tensor_scalar(am[:, 11:12], am[:, 11:12], 1.0, 0.0, Alu.max, Alu.add)
            V.reciprocal(am[:, 10:11], am[:, 11:12])
            V.tensor_tensor(am[:, 0:3], am[:, 0:3], fbcast(am[:, 10:11], 3), Alu.mult)  # mu
            V.tensor_scalar(am[:, 9:10], tot[:, 0:1], -1.0, float(TOPN), Alu.mult, Alu.add)
            V.tensor_tensor(am[:, 0:3], am[:, 0:3], fbcast(am[:, 9:10], 3), Alu.mult)
            V.tensor_tensor(am[:, 0:3], am[:, 0:3], tot[:, 1:4], Alu.add)  # S + amt*mu
            V.tensor_scalar(am[:, 0:3], am[:, 0:3], 1.0 / TOPN, 0.0, Alu.mult, Alu.add)  # Ax
            V.tensor_scalar(am[:, 3:6], am[:, 0:3], 1.0, 1.0, Alu.mult, Alu.add)  # Ax+1
            V.reciprocal(am[:, 3:6], am[:, 3:6])                      # 1/(Ax+1) = 1/(2A)
            V.tensor_scalar(am[:, 0:3], am[:, 0:3], 0.5, 0.5, Alu.mult, Alu.add)  # A img
            V.tensor_scalar(am[:, 6:9], am[:, 0:3], -1.0, 0.5, Alu.mult, Alu.add)  # .5-A
            # broadcast per-channel scalars to [128,1]
            chsc = tiny.tile([128, 9], f32, name="chsc", tag="chsc")
            for k in range(9):
                bp = psml.tile([128, 1], f32, name="ch_ps", tag="fold_ps")
                nc.tensor.matmul(bp[:], c_ones1x[:], am[0:1, k:k + 1], start=True, stop=True)
                nc.scalar.copy(chsc[:, k:k + 1], bp[:])
            # cols 0:3 A_img, 3:6 recipA2, 6:9 bias_d

            # ---- dark2 (bf16) + p ----
            mxp16 = pp.tile([128, NCHUNK * PADW], bf16, name="mxp16", tag="mxp")
            w116 = pp.tile([128, NCHUNK * PADW], bf16, name="w116", tag="w1")
            memset_pads(mxp16, G)
            yr = srcp.tile([128, NCHUNK * CW], bf16, name="yr", tag="srcp")
            yg = srcp.tile([128, NCHUNK * CW], bf16, name="yg", tag="srcp")
            yb_ = srcp.tile([128, NCHUNK * CW], bf16, name="yb", tag="srcp")
            for yt, xt, k in ((yr, xr, 3), (yg, xg, 4), (yb_, xb, 5)):
                nc.scalar.activation(yt[:], xt[:], Act.Identity,
                                     bias=chsc[:, k:k + 1], scale=chsc[:, k:k + 1])
            V.tensor_tensor(interior(mxp16), cview(yr)[:, :, :],
                            cview(yg)[:, :, :], Alu.min)
            V.tensor_tensor(interior(mxp16), interior(mxp16),
                            cview(yb_)[:, :, :], Alu.min)
            uh16 = pp.tile([128, NCHUNK * CW], bf16, name="uh16", tag="uh")
            hpool(uh16, mxp16, w116, V)
            sh16 = pp.tile([128, NCHUNK * CW], bf16, name="sh16", tag="shv")
            u216 = pp.tile([128, NCHUNK * CW], bf16, name="u216", tag="u2")
            vshift_dma(sh16, uh16, 1, c_ones16)
            cmin(u216, uh16, sh16)
            vshift_dma(sh16, u216, 2, c_ones16)
            cmin(uh16, u216, sh16)
            vshift_dma(sh16, uh16, 4, c_ones16)
            cmin(u216, uh16, sh16)
            vshift_dma_down(sh16, u216, 7, c_ones16)
            cmin(uh16, u216, sh16)
            p = srcp.tile([128, NCHUNK * CW], f32, name="p", tag="srcp")
            nc.scalar.activation(p[:], uh16[:], Act.Identity, bias=1.0, scale=-OMEGA)

            # ---- guided filter ----
            Ip = srcp.tile([128, NCHUNK * CW], f32, name="Ip", tag="srcp")
            V.tensor_tensor(Ip[:], guid[:], p[:], Alu.mult)
            II = srcp.tile([128, NCHUNK * CW], f32, name="II", tag="srcp")
            nc.scalar.activation(II[:], guid[:], Act.Square)

            cum = pp.tile([128, NCHUNK * CUMW], f32, name="cum", tag="cum")
            cvz = cview(cum, CUMW)
            G.memset(cvz[:, :, 0:41], 0.0)

            hbs = {}
            for nm, src_t, ee in (("I", guid, V), ("p", p, G), ("Ip", Ip, V),
                                  ("II", II, G)):
                hb_t = boxes.tile([128, NCHUNK * CW], f32r, name=f"hb{nm}", tag="boxes")
                hbox(hb_t, src_t, cum, V, ee)
                hbs[nm] = hb_t
            means = {}
            for nm, ee in (("I", V), ("p", G), ("Ip", V), ("II", G)):
                mn = boxes.tile([128, NCHUNK * CW], f32, name=f"mean{nm}", tag="boxes")
                vbox(mn, hbs[nm], ee)
                means[nm] = mn
            mI, mp_, mIp, mII = means["I"], means["p"], means["Ip"], means["II"]

            tmp = abt.tile([128, NCHUNK * CW], f32, name="tmp", tag="abt")
            G.tensor_tensor(tmp[:], mI[:], mp_[:], Alu.mult)
            cov = abt.tile([128, NCHUNK * CW], f32, name="cov", tag="abt")
            V.tensor_tensor(cov[:], mIp[:], tmp[:], Alu.subtract)
            sq = abt.tile([128, NCHUNK * CW], f32, name="sq", tag="abt")
            nc.scalar.activation(sq[:], mI[:], Act.Square)
            # var + eps = (mII + EPS) - mI^2 in one pass
            V.scalar_tensor_tensor(sq[:], mII[:], EPS, sq[:], Alu.add, Alu.subtract)
            rec = abt.tile([128, NCHUNK * CW], f32, name="rec", tag="abt")
            V.reciprocal_approx_fast(out=rec[:], in_=sq[:])
            a_t = srcp.tile([128, NCHUNK * CW], f32, name="a_t", tag="srcp")
            V.tensor_tensor(a_t[:], cov[:], rec[:], Alu.mult)
            b_t = srcp.tile([128, NCHUNK * CW], f32, name="b_t", tag="srcp")
            G.tensor_tensor(b_t[:], a_t[:], mI[:], Alu.mult)
            G.tensor_tensor(b_t[:], mp_[:], b_t[:], Alu.subtract)

            hba = boxes.tile([128, NCHUNK * CW], f32r, name="hba", tag="boxes")
            hbox(hba, a_t, cum, V, V)
            hbb = boxes.tile([128, NCHUNK * CW], f32r, name="hbb", tag="boxes")
            hbox(hbb, b_t, cum, V, G)
            mean_a = boxes.tile([128, NCHUNK * CW], f32, name="mean_a", tag="boxes")
            vbox(mean_a, hba, V)
            mean_b = boxes.tile([128, NCHUNK * CW], f32, name="mean_b", tag="boxes")
            vbox(mean_b, hbb, G)

            T_t = abt.tile([128, NCHUNK * CW], f32, name="T_t", tag="abt")
            V.tensor_tensor(T_t[:], mean_a[:], guid[:], Alu.mult)
            V.tensor_tensor(T_t[:], T_t[:], mean_b[:], Alu.add)
            rT = abt.tile([128, NCHUNK * CW], f32, name="rT", tag="abt")
            rscr = abt.tile([128, NCHUNK * CW], f32, name="rscr", tag="abt")
            V.reciprocal_approx_accurate(out=rT[:], in_=T_t[:], scratch=rscr[:])

            # ---- final: out_c = (0.5 x_c + (0.5 - A_c)) * rT + A_c ----
            for chn, xt in enumerate((xr, xg, xb)):
                d_t = abt.tile([128, NCHUNK * CW], f32, name=f"d{chn}", tag="dout", bufs=2)
                nc.scalar.activation(d_t[:], xt[:], Act.Identity,
                                     bias=chsc[:, 6 + chn:7 + chn], scale=0.5)
                eng = (V, G, V)[chn]
                eng.tensor_tensor(d_t[:], d_t[:], rT[:], Alu.mult)
                eng.tensor_scalar(d_t[:], d_t[:], chsc[:, chn:chn + 1], 0.0,
                                  Alu.add, Alu.add)
                for c in range(NCHUNK):
                    nc.sync.dma_start(out=y_ext[s, chn, c * 128:(c + 1) * 128, :],
                                      in_=cview(d_t)[:, c, :])

    nc.compile()
    return nc


def _get_program():
    if "nc" not in _CACHE:
        _CACHE["nc"] = _build()
    return _CACHE["nc"]


def kernel(x: np.ndarray) -> np.ndarray:
    from concourse.bass_utils import run_bass_kernel_spmd
    x = np.ascontiguousarray(np.asarray(x, dtype=np.float32))
    assert x.shape == (16, 3, H, W), x.shape
    nc = _get_program()
    consts = _host_consts()
    in_maps = [{"x": x[2 * i:2 * i + 2], **consts} for i in range(8)]
    res = run_bass_kernel_spmd(nc, in_maps, list(range(8)))
    out = np.concatenate([res.results[i]["y"] for i in range(8)], axis=0)
    return out.astype(np.float32)


# revision 20
# speedup vs baseline: 1.2438x; 1.2438x over previous
"""Dark-Channel-Prior dehazing (DCPGenerator) Trainium2 Bass kernel.

Contract: kernel(x: [16,3,512,512] f32) -> [16,3,512,512] f32.
Data-parallel over 8 NeuronCores: 2 samples per core. Each core runs the
full per-sample pipeline on-device:
  guidance/img prep -> dark channel (15x15 min-pool, bf16) -> atmospheric
  light (top-1% selection via secant-estimated threshold + band-corrected
  mean, bf16 counting) -> second dark channel on img/A (bf16) -> guided
  filter (r=40 box sums via free-dim scans + fp32r banded-matmul partition
  sums) -> output.
"""
import numpy as np
from contextlib import ExitStack

H = 512
W = 512
NCHUNK = 4          # 4 row-chunks of 128 partitions
CW = 512            # chunk free width
PADW = 526          # padded chunk width for the 15-wide min pool (7+512+7)
CUMW = 593          # hbox cum chunk: 41 zeros | 512 cumsum | 40 x cum[511]
WIN_PAD = 7
RADIUS = 40
EPS = 1e-3
OMEGA = 0.95
TOPN = int(0.01 * H * W)          # 2621
T0 = 0.0055                       # secant bracket on raw-x dark scale
T1 = 0.0085
BAND = 2e-4                       # band width for tie-region correction
SECANT_ROUNDS = 6

_CACHE = {}


# ---------------------------------------------------------------- host consts
def _host_consts():
    n1 = np.minimum(np.arange(H) + RADIUS, H - 1) - np.maximum(np.arange(H) - RADIUS, 0) + 1
    inv_nh = (1.0 / n1).astype(np.float32)          # [512]
    inv_nw = inv_nh.copy()                          # same for W=512
    invnh = np.zeros((128, NCHUNK), np.float32)
    for c in range(NCHUNK):
        invnh[:, c] = inv_nh[c * 128:(c + 1) * 128]
    invnw_rep = np.broadcast_to(inv_nw[None, :], (128, W)).copy()
    k = np.arange(128)[:, None]
    p = np.arange(128)[None, :]
    band = (np.abs(k - p) <= RADIUS).astype(np.float32)
    bu = (k >= p + 128 - RADIUS).astype(np.float32) / 81.0
    bd = (k <= p - (128 - RADIUS)).astype(np.float32) / 81.0
    bms = []
    for c in range(NCHUNK):
        bms.append(band * inv_nh[c * 128:(c + 1) * 128][None, :] / 81.0)
    bu = bu / 81.0
    bd = bd / 81.0
    # 81*invnw fix factors for the 40 edge columns each side (1.0 interior)
    fixl = np.broadcast_to((81.0 * inv_nw[0:RADIUS])[None, :], (128, RADIUS)).copy()
    fixr = np.broadcast_to((81.0 * inv_nw[W - RADIUS:])[None, :], (128, RADIUS)).copy()
    return {"invnh": invnh, "invnw": invnw_rep, "fixl": fixl, "fixr": fixr,
            "bm0": bms[0], "bm1": bms[1], "bm3": bms[3], "bu": bu, "bd": bd}


# ------------------------------------------------------------------ program
def _build():
    import concourse.bacc as bacc
    import concourse.tile as tile
    import concourse.bass as bass
    from concourse import mybir

    f32 = mybir.dt.float32
    f32r = mybir.dt.float32r
    bf16 = mybir.dt.bfloat16
    Alu = mybir.AluOpType
    Act = mybir.ActivationFunctionType

    nc = bacc.Bacc("TRN2", target_bir_lowering=False, debug=False, num_devices=8)

    x_ext = nc.dram_tensor("x", [2, 3, H, W], f32, kind="ExternalInput").ap()
    band_exts = {nm: nc.dram_tensor(nm, [128, 128], f32, kind="ExternalInput").ap()
                 for nm in ("bm0", "bm1", "bm3", "bu", "bd")}
    invnh_ext = nc.dram_tensor("invnh", [128, NCHUNK], f32, kind="ExternalInput").ap()
    invnw_ext = nc.dram_tensor("invnw", [128, W], f32, kind="ExternalInput").ap()
    fixl_ext = nc.dram_tensor("fixl", [128, RADIUS], f32, kind="ExternalInput").ap()
    fixr_ext = nc.dram_tensor("fixr", [128, RADIUS], f32, kind="ExternalInput").ap()
    y_ext = nc.dram_tensor("y", [2, 3, H, W], f32, kind="ExternalOutput").ap()

    def cview(t, width=CW):
        """[128, NCHUNK*width] tile -> [128, NCHUNK, width] view."""
        return t.rearrange("p (c w) -> p c w", w=width)

    def fbcast(ap_col, n):
        """free-dim step-0 broadcast of a [...,1] AP to [...,n]."""
        return bass.AP(tensor=ap_col.tensor, offset=ap_col.offset,
                       ap=[list(p) for p in ap_col.ap[:-1]] + [[0, n]])

    with ExitStack() as ctx:
        tc = ctx.enter_context(tile.TileContext(nc))

        cpool = ctx.enter_context(tc.tile_pool(name="cpool", bufs=1))
        big = ctx.enter_context(tc.tile_pool(name="big", bufs=1))
        pp = ctx.enter_context(tc.tile_pool(name="pp", bufs=1))       # minpool / box scratch
        boxes = ctx.enter_context(tc.tile_pool(name="boxes", bufs=5))
        srcp = ctx.enter_context(tc.tile_pool(name="srcp", bufs=3))
        abt = ctx.enter_context(tc.tile_pool(name="abt", bufs=3))
        tiny = ctx.enter_context(tc.tile_pool(name="tiny", bufs=2))
        pbig = ctx.enter_context(tc.tile_pool(name="pbig", bufs=2, space="PSUM"))
        psml = ctx.enter_context(tc.tile_pool(name="psml", bufs=2, space="PSUM"))

        # ---- constants ----
        c_band = {}
        stage = cpool.tile([128, 128], f32, name="s_band")
        for nm in ("bm0", "bm1", "bm3", "bu", "bd"):
            nc.sync.dma_start(out=stage[:], in_=band_exts[nm][:])
            c_band[nm] = cpool.tile([128, 128], f32r, name=f"c_{nm}")
            nc.scalar.copy(c_band[nm][:], stage[:])
        c_bm = [c_band["bm0"], c_band["bm1"], c_band["bm1"], c_band["bm3"]]
        c_invnh = cpool.tile([128, NCHUNK], f32, name="c_invnh")
        nc.sync.dma_start(out=c_invnh[:], in_=invnh_ext[:])
        c_invnw = cpool.tile([128, W], f32, name="c_invnw")
        nc.sync.dma_start(out=c_invnw[:], in_=invnw_ext[:])
        c_fixl = cpool.tile([128, RADIUS], f32, name="c_fixl")
        nc.sync.dma_start(out=c_fixl[:], in_=fixl_ext[:])
        c_fixr = cpool.tile([128, RADIUS], f32, name="c_fixr")
        nc.sync.dma_start(out=c_fixr[:], in_=fixr_ext[:])
        c_ones128 = cpool.tile([128, 1], f32, name="c_ones128")
        nc.vector.memset(c_ones128[:], 1.0)
        c_ones1x = cpool.tile([1, 128], f32, name="c_ones1x")
        nc.vector.memset(c_ones1x[:], 1.0)
        c_zeros = cpool.tile([128, CW], f32, name="c_zeros")
        nc.vector.memset(c_zeros[:], 0.0)
        c_ones16 = cpool.tile([128, CW], bf16, name="c_ones16")
        nc.vector.memset(c_ones16[:], 1.0)

        # ------------------------------------------------ helpers (emit ops)
        def interior(t):
            """padded tile -> [128, NCHUNK, CW] strided view of the interiors."""
            return cview(t, PADW)[:, :, WIN_PAD:WIN_PAD + CW]

        def memset_pads(t, eng):
            v = cview(t, PADW)
            for c in range(NCHUNK):
                eng.memset(v[:, c, 0:WIN_PAD], 1.0)
                eng.memset(v[:, c, PADW - WIN_PAD:PADW], 1.0)

        def hpool(dst, padded, w1, eng):
            """15-wide sliding min along free dim; padded [128,4*526] -> dst [128,4*512].
            Single multi-dim-AP instructions across all 4 chunks."""
            a = cview(padded, PADW)
            b = cview(w1, PADW)
            d = cview(dst)
            eng.tensor_tensor(b[:, :, 0:525], a[:, :, 0:525], a[:, :, 1:526], Alu.min)
            eng.tensor_tensor(a[:, :, 0:523], b[:, :, 0:523], b[:, :, 2:525], Alu.min)
            eng.tensor_tensor(b[:, :, 0:519], a[:, :, 0:519], a[:, :, 4:523], Alu.min)
            eng.tensor_tensor(d[:, :, :], b[:, :, 0:512], b[:, :, 7:519], Alu.min)

        def vshift_dma(dst, src, s, pad_tile, ring):
            """dst[row r] = src[row r+s] (global 512-row space), bottom s rows from pad."""
            dv, sv = cview(dst), cview(src)
            ring.dma_start(out=dv[0:128 - s, :, :], in_=sv[s:128, :, :])
            ring.dma_start(out=dv[128 - s:128, 0:NCHUNK - 1, :],
                           in_=sv[0:s, 1:NCHUNK, :])
            ring.dma_start(out=dv[128 - s:128, NCHUNK - 1, :], in_=pad_tile[0:s, :])

        def vshift_dma_down(dst, src, s, pad_tile, ring):
            """dst[row r] = src[max(r-s, 0)] (clamped at the top edge)."""
            dv, sv = cview(dst), cview(src)
            ring.dma_start(out=dv[s:128, :, :], in_=sv[0:128 - s, :, :])
            ring.dma_start(out=dv[0:s, 1:NCHUNK, :],
                           in_=sv[128 - s:128, 0:NCHUNK - 1, :])
            for k in range(s):
                ring.dma_start(out=dv[k:k + 1, 0, :], in_=sv[0:1, 0, :])

        def hbox(dst, src, cum, eng_scan, eng_elem):
            """zero-padded 81-wide box sum along free dim. src [128,2048];
            dst [128,2048] (f32r); cum [128, 4*593] with per-chunk layout
            [41 zeros|512 cum|40 rep] (zeros pre-set once per sample)."""
            sv, dv, cv = cview(src), cview(dst), cview(cum, CUMW)
            for c in range(NCHUNK):
                eng_scan.tensor_tensor_scan(cv[:, c, 41:553], sv[:, c, :], c_zeros[:],
                                            0.0, Alu.add, Alu.add)
            for c in range(NCHUNK):
                eng_elem.tensor_copy(cv[:, c, 553:593], fbcast(cv[:, c, 552:553], 40))
            eng_elem.tensor_tensor(dv[:, :, :], cv[:, :, 81:593], cv[:, :, 0:512],
                                   Alu.subtract)

        def vbox(dst, src, eng=None):
            """mean over the 81-tall zero-padded vertical box. Band matrices
            carry invNh/81; PSUM evacuated by the Scalar engine (plain copy),
            then V fixes the 40 edge columns each side with 81*invNw."""
            sv, dv = cview(src), cview(dst)
            for c in range(NCHUNK):
                ops = []
                if c > 0:
                    ops.append((c_band["bu"], c - 1))
                ops.append((c_bm[c], c))
                if c < NCHUNK - 1:
                    ops.append((c_band["bd"], c + 1))
                ps = pbig.tile([128, CW], f32, name="vps", tag="vps")
                for i, (mat, sc_) in enumerate(ops):
                    nc.tensor.matmul(ps[:], mat[:], sv[:, sc_, :],
                                     start=(i == 0), stop=(i == len(ops) - 1))
                nc.scalar.copy(dv[:, c, :], ps[:])
            nc.vector.tensor_tensor(dv[:, :, 0:RADIUS], dv[:, :, 0:RADIUS],
                                    c_fixl[:].unsqueeze(1).broadcast_to(
                                        [128, NCHUNK, RADIUS]), Alu.mult)
            nc.vector.tensor_tensor(dv[:, :, CW - RADIUS:CW],
                                    dv[:, :, CW - RADIUS:CW],
                                    c_fixr[:].unsqueeze(1).broadcast_to(
                                        [128, NCHUNK, RADIUS]), Alu.mult)

        # ======================================================== per sample
        for s in range(2):
            V = nc.vector
            G = nc.vector  # gpsimd measured ~6x slower on full tiles; keep off path

            # ---- load (one DMA per channel via 3D AP) ----
            xch = []
            for chn in range(3):
                t = big.tile([128, NCHUNK * CW], f32, name=f"x{chn}", tag=f"x{chn}")
                nc.scalar.dma_start(out=cview(t)[:, :, :],
                                    in_=x_ext[s, chn].rearrange("(c p) w -> p c w",
                                                                p=128))
                xch.append(t)
            xr, xg, xb = xch

            # ---- bf16 channel copies (for dark1 + masked sums) ----
            x16 = []
            for chn, xt in enumerate(xch):
                t16 = srcp.tile([128, NCHUNK * CW], bf16, name=f"x16_{chn}",
                                tag="srcp")
                nc.scalar.activation(t16[:], xt[:], Act.Copy, bias=0.0, scale=1.0)
                x16.append(t16)
            xr16, xg16, xb16 = x16

            # ---- guidance I = ((.2989 xr + .587 xg + .114 xb) + 1)/2 (f32) ----
            Ia = pp.tile([128, NCHUNK * CW], f32, name="Ia", tag="shv")
            Ib = pp.tile([128, NCHUNK * CW], f32, name="Ib", tag="Ib")
            nc.scalar.activation(Ib[:], xr[:], Act.Copy, bias=0.5, scale=0.14945)
            V.scalar_tensor_tensor(Ia[:], xg[:], 0.2935, Ib[:], Alu.mult, Alu.add)
            V.scalar_tensor_tensor(Ib[:], xb[:], 0.057, Ia[:], Alu.mult, Alu.add)
            guid = Ib  # final guidance lives in Ib; Ia free for reuse

            # ---- dark1 = minpool15(min_c x) in bf16 ----
            mxp = pp.tile([128, NCHUNK * PADW], bf16, name="mxp", tag="mxp")
            w1 = pp.tile([128, NCHUNK * PADW], bf16, name="w1", tag="w1")
            memset_pads(mxp, V)
            V.tensor_tensor(interior(mxp), cview(xr16)[:, :, :],
                            cview(xg16)[:, :, :], Alu.min)
            V.tensor_tensor(interior(mxp), interior(mxp),
                            cview(xb16)[:, :, :], Alu.min)
            uh = pp.tile([128, NCHUNK * CW], bf16, name="uh", tag="uh")
            hpool(uh, mxp, w1, V)
            sh = pp.tile([128, NCHUNK * CW], bf16, name="sh", tag="shv")
            u2 = pp.tile([128, NCHUNK * CW], bf16, name="u2", tag="u2")

            def cmin(dst_t, a_t2, b_t2, eng=V):
                eng.tensor_tensor(dst_t[:], a_t2[:], b_t2[:], Alu.min)

            vshift_dma(sh, uh, 1, c_ones16, nc.sync)
            cmin(u2, uh, sh)
            vshift_dma(sh, u2, 2, c_ones16, nc.sync)
            cmin(uh, u2, sh)
            vshift_dma(sh, uh, 4, c_ones16, nc.sync)
            cmin(u2, uh, sh)
            vshift_dma_down(sh, u2, 7, c_ones16, nc.sync)
            u = uh
            cmin(u, u2, sh)

            # ---- atmospheric light (bf16 counting / masked sums) ----
            junk = pp.tile([128, NCHUNK * CW], bf16, name="junk", tag="mxp")
            acc8 = tiny.tile([128, 8], f32, name="acc8", tag="acc8")
            V.memset(acc8[:], 0.0)
            thr = tiny.tile([128, 1], f32, name="thr", tag="thr")
            scal = tiny.tile([1, 16], f32, name="scal", tag="scal")
            V.memset(scal[:], 0.0)
            # scal cols: 0 ta, 1 Ca, 2 tb, 3 Cb, 4..temp
            V.memset(scal[:, 0:1], T0)
            V.memset(scal[:, 2:3], T1)

            def count_into(col):
                V.tensor_scalar(junk[:], u[:], thr[:], 0.0, Alu.is_gt, Alu.add,
                                accum_out=acc8[:, col:col + 1])
                fps = psml.tile([1, 1], f32, name="fold_ps", tag="fold_ps")
                nc.tensor.matmul(fps[:], c_ones128[:], acc8[:, col:col + 1],
                                 start=True, stop=True)
                return fps

            def bcast_thr(src_col):
                bp = psml.tile([128, 1], f32, name="thr_ps", tag="fold_ps")
                nc.tensor.matmul(bp[:], c_ones1x[:], src_col, start=True, stop=True)
                nc.scalar.copy(thr[:], bp[:])

            # C(t0), C(t1)
            bcast_thr(scal[0:1, 0:1])
            f = count_into(0)
            nc.scalar.copy(scal[:, 1:2], f[:])
            bcast_thr(scal[0:1, 2:3])
            f = count_into(0)
            nc.scalar.copy(scal[:, 3:4], f[:])
            for _rnd in range(SECANT_ROUNDS):
                # count is monotone non-increasing in t, so sign(dC) = -sign(dT);
                # step = (R - Cb) * dT/dC = (Cb - R) * |dT| / max(|dC|, 1)
                V.tensor_tensor(scal[:, 4:5], scal[:, 3:4], scal[:, 1:2], Alu.subtract)
                V.tensor_scalar(scal[:, 8:9], scal[:, 4:5], -1.0, 0.0, Alu.mult, Alu.add)
                V.tensor_tensor(scal[:, 4:5], scal[:, 4:5], scal[:, 8:9], Alu.max)
                V.tensor_scalar(scal[:, 4:5], scal[:, 4:5], 1.0, 0.0, Alu.max, Alu.add)
                V.tensor_tensor(scal[:, 5:6], scal[:, 2:3], scal[:, 0:1], Alu.subtract)
                V.tensor_scalar(scal[:, 8:9], scal[:, 5:6], -1.0, 0.0, Alu.mult, Alu.add)
                V.tensor_tensor(scal[:, 5:6], scal[:, 5:6], scal[:, 8:9], Alu.max)
                V.reciprocal(scal[:, 8:9], scal[:, 4:5])
                V.tensor_tensor(scal[:, 5:6], scal[:, 5:6], scal[:, 8:9], Alu.mult)
                V.tensor_scalar(scal[:, 6:7], scal[:, 3:4], 1.0, -float(TOPN),
                                Alu.mult, Alu.add)
                V.tensor_tensor(scal[:, 6:7], scal[:, 6:7], scal[:, 5:6], Alu.mult)
                V.tensor_copy(scal[:, 0:1], scal[:, 2:3])
                V.tensor_copy(scal[:, 1:2], scal[:, 3:4])
                V.tensor_tensor(scal[:, 2:3], scal[:, 2:3], scal[:, 6:7], Alu.add)
                bcast_thr(scal[0:1, 2:3])
                f = count_into(0)
                nc.scalar.copy(scal[:, 3:4], f[:])
            # final: C* already in acc8 col0 / scal3. masked sums at thr.
            for chn, xt in enumerate(x16):
                V.scalar_tensor_tensor(junk[:], u[:], thr[:], xt[:], Alu.is_gt,
                                       Alu.mult, accum_out=acc8[:, 1 + chn:2 + chn])
            # band threshold = thr - BAND
            V.tensor_scalar(scal[:, 7:8], scal[:, 2:3], 1.0, -BAND, Alu.mult, Alu.add)
            bcast_thr(scal[0:1, 7:8])
            V.tensor_scalar(junk[:], u[:], thr[:], 0.0, Alu.is_gt, Alu.add,
                            accum_out=acc8[:, 4:5])
            for chn, xt in enumerate(x16):
                V.scalar_tensor_tensor(junk[:], u[:], thr[:], xt[:], Alu.is_gt,
                                       Alu.mult, accum_out=acc8[:, 5 + chn:6 + chn])
            tps = psml.tile([1, 8], f32, name="tot_ps", tag="fold_ps")
            nc.tensor.matmul(tps[:], c_ones128[:], acc8[:], start=True, stop=True)
            tot = tiny.tile([1, 8], f32, name="tot", tag="tot")
            nc.scalar.copy(tot[:], tps[:])
            # A math: tot = [C*, Sr, Sg, Sb, Cb, Sbr, Sbg, Sbb]
            am = tiny.tile([1, 12], f32, name="am", tag="am")
            # am cols: 0:3 A_img, 3:6 recipA2, 6:9 bias_d (0.5-A), 9 amt, 10 recdc, 11 tmp
            V.tensor_tensor(am[:, 0:3], tot[:, 5:8], tot[:, 1:4], Alu.subtract)  # dS
            V.tensor_tensor(am[:, 11:12], tot[:, 4:5], tot[:, 0:1], Alu.subtract)  # dC
            V.tensor_scalar(am[:, 11:12], am[:, 11:12], 1.0, 0.0, Alu.max, Alu.add)
            V.reciprocal(am[:, 10:11], am[:, 11:12])
            V.tensor_tensor(am[:, 0:3], am[:, 0:3], fbcast(am[:, 10:11], 3), Alu.mult)  # mu
            V.tensor_scalar(am[:, 9:10], tot[:, 0:1], -1.0, float(TOPN), Alu.mult, Alu.add)
            V.tensor_tensor(am[:, 0:3], am[:, 0:3], fbcast(am[:, 9:10], 3), Alu.mult)
            V.tensor_tensor(am[:, 0:3], am[:, 0:3], tot[:, 1:4], Alu.add)  # S + amt*mu
            V.tensor_scalar(am[:, 0:3], am[:, 0:3], 1.0 / TOPN, 0.0, Alu.mult, Alu.add)  # Ax
            V.tensor_scalar(am[:, 3:6], am[:, 0:3], 1.0, 1.0, Alu.mult, Alu.add)  # Ax+1
            V.reciprocal(am[:, 3:6], am[:, 3:6])                      # 1/(Ax+1) = 1/(2A)
            V.tensor_scalar(am[:, 0:3], am[:, 0:3], 0.5, 0.5, Alu.mult, Alu.add)  # A img
            V.tensor_scalar(am[:, 6:9], am[:, 0:3], -1.0, 0.5, Alu.mult, Alu.add)  # .5-A
            # broadcast per-channel scalars to [128,1]
            chsc = tiny.tile([128, 9], f32, name="chsc", tag="chsc")
            for k in range(9):
                bp = psml.tile([128, 1], f32, name="ch_ps", tag="fold_ps")
                nc.tensor.matmul(bp[:], c_ones1x[:], am[0:1, k:k + 1], start=True, stop=True)
                nc.scalar.copy(chsc[:, k:k + 1], bp[:])
            # cols 0:3 A_img, 3:6 recipA2, 6:9 bias_d

            # ---- dark2 (bf16) + p ----
            mxp16 = pp.tile([128, NCHUNK * PADW], bf16, name="mxp16", tag="mxp")
            w116 = pp.tile([128, NCHUNK * PADW], bf16, name="w116", tag="w1")
            memset_pads(mxp16, V)
            yr = srcp.tile([128, NCHUNK * CW], bf16, name="yr", tag="srcp")
            yg = srcp.tile([128, NCHUNK * CW], bf16, name="yg", tag="srcp")
            yb_ = srcp.tile([128, NCHUNK * CW], bf16, name="yb", tag="srcp")
            for yt, xt, k in ((yr, xr, 3), (yg, xg, 4), (yb_, xb, 5)):
                nc.scalar.activation(yt[:], xt[:], Act.Identity,
                                     bias=chsc[:, k:k + 1], scale=chsc[:, k:k + 1])
            V.tensor_tensor(interior(mxp16), cview(yr)[:, :, :],
                            cview(yg)[:, :, :], Alu.min)
            V.tensor_tensor(interior(mxp16), interior(mxp16),
                            cview(yb_)[:, :, :], Alu.min)
            uh16 = pp.tile([128, NCHUNK * CW], bf16, name="uh16", tag="uh")
            hpool(uh16, mxp16, w116, V)
            sh16 = pp.tile([128, NCHUNK * CW], bf16, name="sh16", tag="shv")
            u216 = pp.tile([128, NCHUNK * CW], bf16, name="u216", tag="u2")
            vshift_dma(sh16, uh16, 1, c_ones16, nc.sync)
            cmin(u216, uh16, sh16)
            vshift_dma(sh16, u216, 2, c_ones16, nc.sync)
            cmin(uh16, u216, sh16)
            vshift_dma(sh16, uh16, 4, c_ones16, nc.sync)
            cmin(u216, uh16, sh16)
            vshift_dma_down(sh16, u216, 7, c_ones16, nc.sync)
            cmin(uh16, u216, sh16)
            p = srcp.tile([128, NCHUNK * CW], f32, name="p", tag="srcp")
            nc.scalar.activation(p[:], uh16[:], Act.Identity, bias=1.0, scale=-OMEGA)

            # ---- guided filter ----
            Ip = srcp.tile([128, NCHUNK * CW], f32, name="Ip", tag="srcp")
            V.tensor_tensor(Ip[:], guid[:], p[:], Alu.mult)
            II = srcp.tile([128, NCHUNK * CW], f32, name="II", tag="srcp")
            nc.scalar.activation(II[:], guid[:], Act.Square)

            cum = pp.tile([128, NCHUNK * CUMW], f32, name="cum", tag="cum")
            cvz = cview(cum, CUMW)
            for c in range(NCHUNK):
                V.memset(cvz[:, c, 0:41], 0.0)

            hbs = {}
            for nm, src_t, ee in (("I", guid, V), ("p", p, G), ("Ip", Ip, V),
                                  ("II", II, G)):
                hb_t = boxes.tile([128, NCHUNK * CW], f32r, name=f"hb{nm}", tag="boxes")
                hbox(hb_t, src_t, cum, V, ee)
                hbs[nm] = hb_t
            means = {}
            for nm, ee in (("I", V), ("p", G), ("Ip", V), ("II", G)):
                mn = boxes.tile([128, NCHUNK * CW], f32, name=f"mean{nm}", tag="boxes")
                vbox(mn, hbs[nm], ee)
                means[nm] = mn
            mI, mp_, mIp, mII = means["I"], means["p"], means["Ip"], means["II"]

            tmp = abt.tile([128, NCHUNK * CW], f32, name="tmp", tag="abt")
            G.tensor_tensor(tmp[:], mI[:], mp_[:], Alu.mult)
            cov = abt.tile([128, NCHUNK * CW], f32, name="cov", tag="abt")
            V.tensor_tensor(cov[:], mIp[:], tmp[:], Alu.subtract)
            sq = abt.tile([128, NCHUNK * CW], f32, name="sq", tag="abt")
            nc.scalar.activation(sq[:], mI[:], Act.Square)
            # var + eps = (mII + EPS) - mI^2 in one pass
            V.scalar_tensor_tensor(sq[:], mII[:], EPS, sq[:], Alu.add, Alu.subtract)
            rec = abt.tile([128, NCHUNK * CW], f32, name="rec", tag="abt")
            V.reciprocal_approx_fast(out=rec[:], in_=sq[:])
            a_t = srcp.tile([128, NCHUNK * CW], f32, name="a_t", tag="srcp")
            V.tensor_tensor(a_t[:], cov[:], rec[:], Alu.mult)
            b_t = srcp.tile([128, NCHUNK * CW], f32, name="b_t", tag="srcp")
            G.tensor_tensor(b_t[:], a_t[:], mI[:], Alu.mult)
            G.tensor_tensor(b_t[:], mp_[:], b_t[:], Alu.subtract)

            hba = boxes.tile([128, NCHUNK * CW], f32r, name="hba", tag="boxes")
            hbox(hba, a_t, cum, V, V)
            hbb = boxes.tile([128, NCHUNK * CW], f32r, name="hbb", tag="boxes")
            hbox(hbb, b_t, cum, V, G)
            mean_a = boxes.tile([128, NCHUNK * CW], f32, name="mean_a", tag="boxes")
            vbox(mean_a, hba, V)
            mean_b = boxes.tile([128, NCHUNK * CW], f32, name="mean_b", tag="boxes")
            vbox(mean_b, hbb, G)

            T_t = abt.tile([128, NCHUNK * CW], f32, name="T_t", tag="abt")
            V.tensor_tensor(T_t[:], mean_a[:], guid[:], Alu.mult)
            V.tensor_tensor(T_t[:], T_t[:], mean_b[:], Alu.add)
            rT = abt.tile([128, NCHUNK * CW], f32, name="rT", tag="abt")
            rscr = abt.tile([128, NCHUNK * CW], f32, name="rscr", tag="abt")
            V.reciprocal_approx_accurate(out=rT[:], in_=T_t[:], scratch=rscr[:])

            # ---- final: out_c = (0.5 x_c + (0.5 - A_c)) * rT + A_c ----
            for chn, xt in enumerate((xr, xg, xb)):
                d_t = abt.tile([128, NCHUNK * CW], f32, name=f"d{chn}", tag="dout", bufs=2)
                nc.scalar.activation(d_t[:], xt[:], Act.Identity,
                                     bias=chsc[:, 6 + chn:7 + chn], scale=0.5)
                V.tensor_tensor(d_t[:], d_t[:], rT[:], Alu.mult)
                V.tensor_scalar(d_t[:], d_t[:], chsc[:, chn:chn + 1], 0.0,
                                Alu.add, Alu.add)
                nc.scalar.dma_start(out=y_ext[s, chn].rearrange("(c p) w -> p c w",
                                                                p=128),
                                    in_=cview(d_t)[:, :, :])

    nc.compile()
    return nc


def _get_program():
    if "nc" not in _CACHE:
        _CACHE["nc"] = _build()
    return _CACHE["nc"]


def kernel(x: np.ndarray) -> np.ndarray:
    from concourse.bass_utils import run_bass_kernel_spmd
    x = np.ascontiguousarray(np.asarray(x, dtype=np.float32))
    assert x.shape == (16, 3, H, W), x.shape
    nc = _get_program()
    consts = _host_consts()
    in_maps = [{"x": x[2 * i:2 * i + 2], **consts} for i in range(8)]
    res = run_bass_kernel_spmd(nc, in_maps, list(range(8)))
    out = np.concatenate([res.results[i]["y"] for i in range(8)], axis=0)
    return out.astype(np.float32)


# revision 31
# speedup vs baseline: 1.3773x; 1.1073x over previous
"""Dark-Channel-Prior dehazing (DCPGenerator) Trainium2 Bass kernel.

Contract: kernel(x: [16,3,512,512] f32) -> [16,3,512,512] f32.
Data-parallel over 8 NeuronCores: 2 samples per core. Each core runs the
full per-sample pipeline on-device:
  guidance/img prep -> dark channel (15x15 min-pool, bf16) -> atmospheric
  light (top-1% selection via secant-estimated threshold + band-corrected
  mean, bf16 counting) -> second dark channel on img/A (bf16) -> guided
  filter (r=40 box sums via free-dim scans + fp32r banded-matmul partition
  sums) -> output.
"""
import numpy as np
from contextlib import ExitStack

H = 512
W = 512
NCHUNK = 4          # 4 row-chunks of 128 partitions
CW = 512            # chunk free width
PADW = 526          # padded chunk width for the 15-wide min pool (7+512+7)
CUMW = 593          # hbox cum chunk: 41 zeros | 512 cumsum | 40 x cum[511]
WIN_PAD = 7
RADIUS = 40
EPS = 1e-3
OMEGA = 0.95
TOPN = int(0.01 * H * W)          # 2621
T0 = 0.0055                       # secant bracket on raw-x dark scale
T1 = 0.0085
BAND = 2e-4                       # band width for tie-region correction
SECANT_ROUNDS = 6

_CACHE = {}


# ---------------------------------------------------------------- host consts
def _host_consts():
    n1 = np.minimum(np.arange(H) + RADIUS, H - 1) - np.maximum(np.arange(H) - RADIUS, 0) + 1
    inv_nh = (1.0 / n1).astype(np.float32)          # [512]
    inv_nw = inv_nh.copy()                          # same for W=512
    invnh = np.zeros((128, NCHUNK), np.float32)
    for c in range(NCHUNK):
        invnh[:, c] = inv_nh[c * 128:(c + 1) * 128]
    invnw_rep = np.broadcast_to(inv_nw[None, :], (128, W)).copy()
    k = np.arange(128)[:, None]
    p = np.arange(128)[None, :]
    band = (np.abs(k - p) <= RADIUS).astype(np.float32)
    bu = (k >= p + 128 - RADIUS).astype(np.float32) / 81.0
    bd = (k <= p - (128 - RADIUS)).astype(np.float32) / 81.0
    bms = []
    for c in range(NCHUNK):
        bms.append(band * inv_nh[c * 128:(c + 1) * 128][None, :] / 81.0)
    bu = bu / 81.0
    bd = bd / 81.0
    # 81*invnw fix factors for the 40 edge columns each side (1.0 interior)
    fixl = np.broadcast_to((81.0 * inv_nw[0:RADIUS])[None, :], (128, RADIUS)).copy()
    fixr = np.broadcast_to((81.0 * inv_nw[W - RADIUS:])[None, :], (128, RADIUS)).copy()
    return {"invnh": invnh, "invnw": invnw_rep, "fixl": fixl, "fixr": fixr,
            "bm0": bms[0], "bm1": bms[1], "bm3": bms[3], "bu": bu, "bd": bd}


# ------------------------------------------------------------------ program
def _build():
    import concourse.bacc as bacc
    import concourse.tile as tile
    import concourse.bass as bass
    from concourse import mybir

    f32 = mybir.dt.float32
    f32r = mybir.dt.float32r
    bf16 = mybir.dt.bfloat16
    Alu = mybir.AluOpType
    Act = mybir.ActivationFunctionType

    nc = bacc.Bacc("TRN2", target_bir_lowering=False, debug=False, num_devices=8)

    x_ext = nc.dram_tensor("x", [2, 3, H, W], f32, kind="ExternalInput").ap()
    band_exts = {nm: nc.dram_tensor(nm, [128, 128], f32, kind="ExternalInput").ap()
                 for nm in ("bm0", "bm1", "bm3", "bu", "bd")}
    invnh_ext = nc.dram_tensor("invnh", [128, NCHUNK], f32, kind="ExternalInput").ap()
    invnw_ext = nc.dram_tensor("invnw", [128, W], f32, kind="ExternalInput").ap()
    fixl_ext = nc.dram_tensor("fixl", [128, RADIUS], f32, kind="ExternalInput").ap()
    fixr_ext = nc.dram_tensor("fixr", [128, RADIUS], f32, kind="ExternalInput").ap()
    y_ext = nc.dram_tensor("y", [2, 3, H, W], f32, kind="ExternalOutput").ap()

    def cview(t, width=CW):
        """[128, NCHUNK*width] tile -> [128, NCHUNK, width] view."""
        return t.rearrange("p (c w) -> p c w", w=width)

    def fbcast(ap_col, n):
        """free-dim step-0 broadcast of a [...,1] AP to [...,n]."""
        return bass.AP(tensor=ap_col.tensor, offset=ap_col.offset,
                       ap=[list(p) for p in ap_col.ap[:-1]] + [[0, n]])

    with ExitStack() as ctx:
        tc = ctx.enter_context(tile.TileContext(nc))

        cpool = ctx.enter_context(tc.tile_pool(name="cpool", bufs=1))
        big = ctx.enter_context(tc.tile_pool(name="big", bufs=1))
        pp = ctx.enter_context(tc.tile_pool(name="pp", bufs=1))       # minpool / box scratch
        boxes = ctx.enter_context(tc.tile_pool(name="boxes", bufs=5))
        srcp = ctx.enter_context(tc.tile_pool(name="srcp", bufs=3))
        abt = ctx.enter_context(tc.tile_pool(name="abt", bufs=3))
        tiny = ctx.enter_context(tc.tile_pool(name="tiny", bufs=2))
        pbig = ctx.enter_context(tc.tile_pool(name="pbig", bufs=2, space="PSUM"))
        psml = ctx.enter_context(tc.tile_pool(name="psml", bufs=2, space="PSUM"))

        # ---- constants ----
        c_band = {}
        stage = cpool.tile([128, 128], f32, name="s_band")
        for nm in ("bm0", "bm1", "bm3", "bu", "bd"):
            nc.sync.dma_start(out=stage[:], in_=band_exts[nm][:])
            c_band[nm] = cpool.tile([128, 128], f32r, name=f"c_{nm}")
            nc.scalar.copy(c_band[nm][:], stage[:])
        c_bm = [c_band["bm0"], c_band["bm1"], c_band["bm1"], c_band["bm3"]]
        c_invnh = cpool.tile([128, NCHUNK], f32, name="c_invnh")
        nc.sync.dma_start(out=c_invnh[:], in_=invnh_ext[:])
        c_invnw = cpool.tile([128, W], f32, name="c_invnw")
        nc.sync.dma_start(out=c_invnw[:], in_=invnw_ext[:])
        c_fixl = cpool.tile([128, RADIUS], f32, name="c_fixl")
        nc.sync.dma_start(out=c_fixl[:], in_=fixl_ext[:])
        c_fixr = cpool.tile([128, RADIUS], f32, name="c_fixr")
        nc.sync.dma_start(out=c_fixr[:], in_=fixr_ext[:])
        c_ones128 = cpool.tile([128, 1], f32, name="c_ones128")
        nc.vector.memset(c_ones128[:], 1.0)
        c_ones1x = cpool.tile([1, 128], f32, name="c_ones1x")
        nc.vector.memset(c_ones1x[:], 1.0)
        c_zeros = cpool.tile([128, CW], f32, name="c_zeros")
        nc.vector.memset(c_zeros[:], 0.0)
        c_ones16 = cpool.tile([128, CW], bf16, name="c_ones16")
        nc.vector.memset(c_ones16[:], 1.0)
        c_e07 = cpool.tile([128, 7], bf16, name="c_e07")
        nc.vector.memset(c_e07[:], 0.0)
        nc.vector.memset(c_e07[0:1, :], 1.0)

        # ------------------------------------------------ helpers (emit ops)
        def interior(t):
            """padded tile -> [128, NCHUNK, CW] strided view of the interiors."""
            return cview(t, PADW)[:, :, WIN_PAD:WIN_PAD + CW]

        def memset_pads(t, eng):
            v = cview(t, PADW)
            for c in range(NCHUNK):
                eng.memset(v[:, c, 0:WIN_PAD], 1.0)
                eng.memset(v[:, c, PADW - WIN_PAD:PADW], 1.0)

        def hpool(dst, padded, w1, eng):
            """15-wide sliding min along free dim; padded [128,4*526] -> dst [128,4*512].
            Single multi-dim-AP instructions across all 4 chunks."""
            a = cview(padded, PADW)
            b = cview(w1, PADW)
            d = cview(dst)
            eng.tensor_tensor(b[:, :, 0:525], a[:, :, 0:525], a[:, :, 1:526], Alu.min)
            eng.tensor_tensor(a[:, :, 0:523], b[:, :, 0:523], b[:, :, 2:525], Alu.min)
            eng.tensor_tensor(b[:, :, 0:519], a[:, :, 0:519], a[:, :, 4:523], Alu.min)
            eng.tensor_tensor(d[:, 0:NCHUNK, :], b[:, :, 0:512], b[:, :, 7:519],
                              Alu.min)

        def vshift_dma(dst, src, s, ring):
            """dst[row r] = src[row r+s] (global 512-row space). src has a
            5th all-ones chunk so the bottom-pad rows ride the wrap DMA."""
            dv, sv = cview(dst), cview(src)
            ring.dma_start(out=dv[0:128 - s, :, :], in_=sv[s:128, 0:NCHUNK, :])
            ring.dma_start(out=dv[128 - s:128, :, :], in_=sv[0:s, 1:NCHUNK + 1, :])

        def vshift_dma_down(dst, src, s, ring):
            """dst[row r] = src[max(r-s, 0)]; the top-clamp rows of chunk 0 are
            NOT written here — the caller patches them via a PE broadcast."""
            dv, sv = cview(dst), cview(src)
            ring.dma_start(out=dv[s:128, :, :], in_=sv[0:128 - s, 0:NCHUNK, :])
            ring.dma_start(out=dv[0:s, 1:NCHUNK, :],
                           in_=sv[128 - s:128, 0:NCHUNK - 1, :])

        def hbox(dst, src, cum, eng_scan, eng_elem):
            """zero-padded 81-wide box sum along free dim. src [128,2048];
            dst [128,2048] (f32r); cum [128, 4*593] with per-chunk layout
            [41 zeros|512 cum|40 rep] (zeros pre-set once per sample)."""
            sv, dv, cv = cview(src), cview(dst), cview(cum, CUMW)
            for c in range(NCHUNK):
                eng_scan.tensor_tensor_scan(cv[:, c, 41:553], sv[:, c, :], c_zeros[:],
                                            0.0, Alu.add, Alu.add)
            for c in range(NCHUNK):
                eng_elem.tensor_copy(cv[:, c, 553:593], fbcast(cv[:, c, 552:553], 40))
            eng_elem.tensor_tensor(dv[:, :, :], cv[:, :, 81:593], cv[:, :, 0:512],
                                   Alu.subtract)

        def vbox(dst, src, eng=None):
            """mean over the 81-tall zero-padded vertical box. Band matrices
            carry invNh/81; PSUM evacuated by the Scalar engine (plain copy),
            then V fixes the 40 edge columns each side with 81*invNw."""
            sv, dv = cview(src), cview(dst)
            for c in range(NCHUNK):
                ops = []
                if c > 0:
                    ops.append((c_band["bu"], c - 1))
                ops.append((c_bm[c], c))
                if c < NCHUNK - 1:
                    ops.append((c_band["bd"], c + 1))
                ps = pbig.tile([128, CW], f32, name="vps", tag="vps")
                for i, (mat, sc_) in enumerate(ops):
                    nc.tensor.matmul(ps[:], mat[:], sv[:, sc_, :],
                                     start=(i == 0), stop=(i == len(ops) - 1))
                nc.scalar.copy(dv[:, c, :], ps[:])
            nc.vector.tensor_tensor(dv[:, :, 0:RADIUS], dv[:, :, 0:RADIUS],
                                    c_fixl[:].unsqueeze(1).broadcast_to(
                                        [128, NCHUNK, RADIUS]), Alu.mult)
            nc.vector.tensor_tensor(dv[:, :, CW - RADIUS:CW],
                                    dv[:, :, CW - RADIUS:CW],
                                    c_fixr[:].unsqueeze(1).broadcast_to(
                                        [128, NCHUNK, RADIUS]), Alu.mult)

        # ======================================================== per sample
        for s in range(2):
            V = nc.vector
            G = nc.vector  # gpsimd measured ~6x slower on full tiles; keep off path

            # ---- load (one DMA per channel via 3D AP) ----
            xch = []
            for chn in range(3):
                t = big.tile([128, NCHUNK * CW], f32, name=f"x{chn}", tag=f"x{chn}")
                nc.scalar.dma_start(out=cview(t)[:, :, :],
                                    in_=x_ext[s, chn].rearrange("(c p) w -> p c w",
                                                                p=128))
                xch.append(t)
            xr, xg, xb = xch

            # ---- bf16 channel copies (for dark1 + masked sums) ----
            x16 = []
            for chn, xt in enumerate(xch):
                t16 = srcp.tile([128, NCHUNK * CW], bf16, name=f"x16_{chn}",
                                tag="srcp")
                nc.scalar.activation(t16[:], xt[:], Act.Copy, bias=0.0, scale=1.0)
                x16.append(t16)
            xr16, xg16, xb16 = x16

            # ---- guidance I = ((.2989 xr + .587 xg + .114 xb) + 1)/2 (f32) ----
            Ia = pp.tile([128, NCHUNK * CW], bf16, name="Ia", tag="shv")
            Ib = pp.tile([128, NCHUNK * CW], bf16, name="Ib", tag="Ib")
            nc.scalar.activation(Ib[:], xr[:], Act.Copy, bias=0.5, scale=0.14945)
            V.scalar_tensor_tensor(Ia[:], xg16[:], 0.2935, Ib[:], Alu.mult, Alu.add)
            V.scalar_tensor_tensor(Ib[:], xb16[:], 0.057, Ia[:], Alu.mult, Alu.add)
            guid = Ib  # final guidance lives in Ib; Ia free for reuse

            # ---- dark1 = minpool15(min_c x) in bf16 ----
            mxp = pp.tile([128, NCHUNK * PADW], bf16, name="mxp", tag="mxp")
            w1 = pp.tile([128, NCHUNK * PADW], bf16, name="w1", tag="w1")
            memset_pads(mxp, V)
            V.tensor_tensor(interior(mxp), cview(xr16)[:, :, :],
                            cview(xg16)[:, :, :], Alu.min)
            V.tensor_tensor(interior(mxp), interior(mxp),
                            cview(xb16)[:, :, :], Alu.min)
            uh = pp.tile([128, (NCHUNK + 1) * CW], bf16, name="uh", tag="uh")
            V.memset(cview(uh)[:, NCHUNK, :], 1.0)
            hpool(uh, mxp, w1, V)
            sh = pp.tile([128, NCHUNK * CW], bf16, name="sh", tag="shv")
            u2 = pp.tile([128, (NCHUNK + 1) * CW], bf16, name="u2", tag="u2")
            V.memset(cview(u2)[:, NCHUNK, :], 1.0)
            NW = NCHUNK * CW

            def cmin(dst_t, a_t2, b_t2, eng=V):
                eng.tensor_tensor(dst_t[:, 0:NW], a_t2[:, 0:NW], b_t2[:, 0:NW],
                                  Alu.min)

            def clamp_fix(dst_t, src_t):
                """dst[0:7, chunk0] = min(src[0:7, chunk0], src[row0, chunk0])."""
                bc = pbig.tile([7, CW], f32, name="clamp_ps", tag="clamp")
                nc.tensor.matmul(bc[:], c_e07[:], cview(src_t)[:, 0, :],
                                 start=True, stop=True)
                V.tensor_tensor(cview(dst_t)[0:7, 0, :], cview(src_t)[0:7, 0, :],
                                bc[:], Alu.min)

            vshift_dma(sh, uh, 1, nc.sync)
            cmin(u2, uh, sh)
            vshift_dma(sh, u2, 2, nc.sync)
            cmin(uh, u2, sh)
            vshift_dma(sh, uh, 4, nc.sync)
            cmin(u2, uh, sh)
            vshift_dma_down(sh, u2, 7, nc.sync)
            u = uh
            cmin(u, u2, sh)
            clamp_fix(u, u2)

            # ---- atmospheric light (bf16 counting / masked sums) ----
            junk = pp.tile([128, NCHUNK * CW], bf16, name="junk", tag="mxp")
            acc8 = tiny.tile([128, 8], f32, name="acc8", tag="acc8")
            V.memset(acc8[:], 0.0)
            thr = tiny.tile([128, 1], f32, name="thr", tag="thr")
            scal = tiny.tile([1, 16], f32, name="scal", tag="scal")
            V.memset(scal[:], 0.0)
            # scal cols: 0 ta, 1 Ca, 2 tb, 3 Cb, 4..temp
            V.memset(scal[:, 0:1], T0)
            V.memset(scal[:, 2:3], T1)

            uv = cview(u)

            def count_into(col, sub=False):
                if sub:
                    # chunks {0,2}, stride-2 cols: 1/4 of the pixels
                    V.tensor_scalar(cview(junk)[:, 0:2, 0:256],
                                    uv[:, 0:NCHUNK:2, 0:CW:2], thr[:], 0.0,
                                    Alu.is_gt, Alu.add,
                                    accum_out=acc8[:, col:col + 1])
                else:
                    V.tensor_scalar(junk[:], u[:, 0:NW], thr[:], 0.0,
                                    Alu.is_gt, Alu.add,
                                    accum_out=acc8[:, col:col + 1])
                fps = psml.tile([1, 1], f32, name="fold_ps", tag="fold_ps")
                nc.tensor.matmul(fps[:], c_ones128[:], acc8[:, col:col + 1],
                                 start=True, stop=True)
                return fps

            def bcast_thr(src_col):
                bp = psml.tile([128, 1], f32, name="thr_ps", tag="fold_ps")
                nc.tensor.matmul(bp[:], c_ones1x[:], src_col, start=True, stop=True)
                nc.scalar.copy(thr[:], bp[:])

            # C(t0), C(t1) on the 1/4 subsample (band-corrected later)
            bcast_thr(scal[0:1, 0:1])
            f = count_into(0, sub=True)
            nc.scalar.copy(scal[:, 1:2], f[:])
            bcast_thr(scal[0:1, 2:3])
            f = count_into(0, sub=True)
            nc.scalar.copy(scal[:, 3:4], f[:])
            for _rnd in range(SECANT_ROUNDS):
                # count is monotone non-increasing in t, so sign(dC) = -sign(dT);
                # step = (R - Cb) * dT/dC = (Cb - R) * |dT| / max(|dC|, 1)
                V.tensor_tensor(scal[:, 4:5], scal[:, 3:4], scal[:, 1:2], Alu.subtract)
                V.tensor_scalar(scal[:, 8:9], scal[:, 4:5], -1.0, 0.0, Alu.mult, Alu.add)
                V.tensor_tensor(scal[:, 4:5], scal[:, 4:5], scal[:, 8:9], Alu.max)
                V.tensor_scalar(scal[:, 4:5], scal[:, 4:5], 1.0, 0.0, Alu.max, Alu.add)
                V.tensor_tensor(scal[:, 5:6], scal[:, 2:3], scal[:, 0:1], Alu.subtract)
                V.tensor_scalar(scal[:, 8:9], scal[:, 5:6], -1.0, 0.0, Alu.mult, Alu.add)
                V.tensor_tensor(scal[:, 5:6], scal[:, 5:6], scal[:, 8:9], Alu.max)
                V.reciprocal(scal[:, 8:9], scal[:, 4:5])
                V.tensor_tensor(scal[:, 5:6], scal[:, 5:6], scal[:, 8:9], Alu.mult)
                V.tensor_scalar(scal[:, 6:7], scal[:, 3:4], 1.0, -TOPN / 4.0,
                                Alu.mult, Alu.add)
                V.tensor_tensor(scal[:, 6:7], scal[:, 6:7], scal[:, 5:6], Alu.mult)
                V.tensor_copy(scal[:, 0:1], scal[:, 2:3])
                V.tensor_copy(scal[:, 1:2], scal[:, 3:4])
                V.tensor_tensor(scal[:, 2:3], scal[:, 2:3], scal[:, 6:7], Alu.add)
                bcast_thr(scal[0:1, 2:3])
                f = count_into(0, sub=True)
                nc.scalar.copy(scal[:, 3:4], f[:])
            # final full-res count C* at thr, then masked sums at thr.
            count_into(0)
            for chn, xt in enumerate(x16):
                V.scalar_tensor_tensor(junk[:], u[:, 0:NW], thr[:], xt[:],
                                       Alu.is_gt, Alu.mult,
                                       accum_out=acc8[:, 1 + chn:2 + chn])
            # band threshold = thr - BAND
            V.tensor_scalar(scal[:, 7:8], scal[:, 2:3], 1.0, -BAND, Alu.mult, Alu.add)
            bcast_thr(scal[0:1, 7:8])
            V.tensor_scalar(junk[:], u[:, 0:NW], thr[:], 0.0, Alu.is_gt, Alu.add,
                            accum_out=acc8[:, 4:5])
            for chn, xt in enumerate(x16):
                V.scalar_tensor_tensor(junk[:], u[:, 0:NW], thr[:], xt[:],
                                       Alu.is_gt, Alu.mult,
                                       accum_out=acc8[:, 5 + chn:6 + chn])
            tps = psml.tile([1, 8], f32, name="tot_ps", tag="fold_ps")
            nc.tensor.matmul(tps[:], c_ones128[:], acc8[:], start=True, stop=True)
            tot = tiny.tile([1, 8], f32, name="tot", tag="tot")
            nc.scalar.copy(tot[:], tps[:])
            # A math: tot = [C*, Sr, Sg, Sb, Cb, Sbr, Sbg, Sbb]
            am = tiny.tile([1, 12], f32, name="am", tag="am")
            # am cols: 0:3 A_img, 3:6 recipA2, 6:9 bias_d (0.5-A), 9 amt, 10 recdc, 11 tmp
            V.tensor_tensor(am[:, 0:3], tot[:, 5:8], tot[:, 1:4], Alu.subtract)  # dS
            V.tensor_tensor(am[:, 11:12], tot[:, 4:5], tot[:, 0:1], Alu.subtract)  # dC
            V.tensor_scalar(am[:, 11:12], am[:, 11:12], 1.0, 0.0, Alu.max, Alu.add)
            V.reciprocal(am[:, 10:11], am[:, 11:12])
            V.tensor_tensor(am[:, 0:3], am[:, 0:3], fbcast(am[:, 10:11], 3), Alu.mult)  # mu
            V.tensor_scalar(am[:, 9:10], tot[:, 0:1], -1.0, float(TOPN), Alu.mult, Alu.add)
            V.tensor_tensor(am[:, 0:3], am[:, 0:3], fbcast(am[:, 9:10], 3), Alu.mult)
            V.tensor_tensor(am[:, 0:3], am[:, 0:3], tot[:, 1:4], Alu.add)  # S + amt*mu
            V.tensor_scalar(am[:, 0:3], am[:, 0:3], 1.0 / TOPN, 0.0, Alu.mult, Alu.add)  # Ax
            V.tensor_scalar(am[:, 3:6], am[:, 0:3], 1.0, 1.0, Alu.mult, Alu.add)  # Ax+1
            V.reciprocal(am[:, 3:6], am[:, 3:6])                      # 1/(Ax+1) = 1/(2A)
            V.tensor_scalar(am[:, 0:3], am[:, 0:3], 0.5, 0.5, Alu.mult, Alu.add)  # A img
            V.tensor_scalar(am[:, 6:9], am[:, 0:3], -1.0, 0.5, Alu.mult, Alu.add)  # .5-A
            # broadcast per-channel scalars to [128,1]
            chsc = tiny.tile([128, 9], f32, name="chsc", tag="chsc")
            for k in range(9):
                bp = psml.tile([128, 1], f32, name="ch_ps", tag="fold_ps")
                nc.tensor.matmul(bp[:], c_ones1x[:], am[0:1, k:k + 1], start=True, stop=True)
                nc.scalar.copy(chsc[:, k:k + 1], bp[:])
            # cols 0:3 A_img, 3:6 recipA2, 6:9 bias_d

            # ---- dark2 (bf16) + p ----
            mxp16 = pp.tile([128, NCHUNK * PADW], bf16, name="mxp16", tag="mxp")
            w116 = pp.tile([128, NCHUNK * PADW], bf16, name="w116", tag="w1")
            memset_pads(mxp16, V)
            yr = srcp.tile([128, NCHUNK * CW], bf16, name="yr", tag="srcp")
            yg = srcp.tile([128, NCHUNK * CW], bf16, name="yg", tag="srcp")
            yb_ = srcp.tile([128, NCHUNK * CW], bf16, name="yb", tag="srcp")
            for yt, xt, k in ((yr, xr, 3), (yg, xg, 4), (yb_, xb, 5)):
                nc.scalar.activation(yt[:], xt[:], Act.Identity,
                                     bias=chsc[:, k:k + 1], scale=chsc[:, k:k + 1])
            V.tensor_tensor(interior(mxp16), cview(yr)[:, :, :],
                            cview(yg)[:, :, :], Alu.min)
            V.tensor_tensor(interior(mxp16), interior(mxp16),
                            cview(yb_)[:, :, :], Alu.min)
            uh16 = pp.tile([128, (NCHUNK + 1) * CW], bf16, name="uh16", tag="uh")
            V.memset(cview(uh16)[:, NCHUNK, :], 1.0)
            hpool(uh16, mxp16, w116, V)
            sh16 = pp.tile([128, NCHUNK * CW], bf16, name="sh16", tag="shv")
            u216 = pp.tile([128, (NCHUNK + 1) * CW], bf16, name="u216", tag="u2")
            V.memset(cview(u216)[:, NCHUNK, :], 1.0)
            vshift_dma(sh16, uh16, 1, nc.sync)
            cmin(u216, uh16, sh16)
            vshift_dma(sh16, u216, 2, nc.sync)
            cmin(uh16, u216, sh16)
            vshift_dma(sh16, uh16, 4, nc.sync)
            cmin(u216, uh16, sh16)
            vshift_dma_down(sh16, u216, 7, nc.sync)
            cmin(uh16, u216, sh16)
            clamp_fix(uh16, u216)
            p = srcp.tile([128, NCHUNK * CW], bf16, name="p", tag="srcp")
            nc.scalar.activation(p[:], uh16[:, 0:NW], Act.Identity, bias=1.0,
                                 scale=-OMEGA)

            # ---- guided filter ----
            Ip = srcp.tile([128, NCHUNK * CW], bf16, name="Ip", tag="srcp")
            V.tensor_tensor(Ip[:], guid[:], p[:], Alu.mult)
            II = srcp.tile([128, NCHUNK * CW], bf16, name="II", tag="srcp")
            nc.scalar.activation(II[:], guid[:], Act.Square)

            cum = pp.tile([128, NCHUNK * CUMW], f32, name="cum", tag="cum")
            cvz = cview(cum, CUMW)
            for c in range(NCHUNK):
                V.memset(cvz[:, c, 0:41], 0.0)

            hbs = {}
            for nm, src_t, ee in (("I", guid, V), ("p", p, G), ("Ip", Ip, V),
                                  ("II", II, G)):
                hb_t = boxes.tile([128, NCHUNK * CW], f32r, name=f"hb{nm}", tag="boxes")
                hbox(hb_t, src_t, cum, V, ee)
                hbs[nm] = hb_t
            means = {}
            for nm, ee in (("I", V), ("p", G), ("Ip", V), ("II", G)):
                mn = boxes.tile([128, NCHUNK * CW], f32, name=f"mean{nm}", tag="boxes")
                vbox(mn, hbs[nm], ee)
                means[nm] = mn
            mI, mp_, mIp, mII = means["I"], means["p"], means["Ip"], means["II"]

            tmp = abt.tile([128, NCHUNK * CW], f32, name="tmp", tag="abt")
            G.tensor_tensor(tmp[:], mI[:], mp_[:], Alu.mult)
            cov = abt.tile([128, NCHUNK * CW], f32, name="cov", tag="abt")
            V.tensor_tensor(cov[:], mIp[:], tmp[:], Alu.subtract)
            sq = abt.tile([128, NCHUNK * CW], f32, name="sq", tag="abt")
            nc.scalar.activation(sq[:], mI[:], Act.Square)
            # var + eps = (mII + EPS) - mI^2 in one pass
            V.scalar_tensor_tensor(sq[:], mII[:], EPS, sq[:], Alu.add, Alu.subtract)
            rec = abt.tile([128, NCHUNK * CW], f32, name="rec", tag="abt")
            V.reciprocal_approx_fast(out=rec[:], in_=sq[:])
            a_t = srcp.tile([128, NCHUNK * CW], f32, name="a_t", tag="srcp")
            V.tensor_tensor(a_t[:], cov[:], rec[:], Alu.mult)
            b_t = srcp.tile([128, NCHUNK * CW], f32, name="b_t", tag="srcp")
            G.tensor_tensor(b_t[:], a_t[:], mI[:], Alu.mult)
            G.tensor_tensor(b_t[:], mp_[:], b_t[:], Alu.subtract)

            hba = boxes.tile([128, NCHUNK * CW], f32r, name="hba", tag="boxes")
            hbox(hba, a_t, cum, V, V)
            hbb = boxes.tile([128, NCHUNK * CW], f32r, name="hbb", tag="boxes")
            hbox(hbb, b_t, cum, V, G)
            mean_a = boxes.tile([128, NCHUNK * CW], f32, name="mean_a", tag="boxes")
            vbox(mean_a, hba, V)
            mean_b = boxes.tile([128, NCHUNK * CW], f32, name="mean_b", tag="boxes")
            vbox(mean_b, hbb, G)

            T_t = abt.tile([128, NCHUNK * CW], f32, name="T_t", tag="abt")
            V.tensor_tensor(T_t[:], mean_a[:], guid[:], Alu.mult)
            V.tensor_tensor(T_t[:], T_t[:], mean_b[:], Alu.add)
            rT = abt.tile([128, NCHUNK * CW], f32, name="rT", tag="abt")
            V.reciprocal_approx_fast(out=rT[:], in_=T_t[:])

            # ---- final: out_c = (0.5 x_c + (0.5 - A_c)) * rT + A_c ----
            for chn, xt in enumerate((xr, xg, xb)):
                d_t = abt.tile([128, NCHUNK * CW], f32, name=f"d{chn}", tag="dout", bufs=2)
                nc.scalar.activation(d_t[:], xt[:], Act.Identity,
                                     bias=chsc[:, 6 + chn:7 + chn], scale=0.5)
                V.tensor_tensor(d_t[:], d_t[:], rT[:], Alu.mult)
                V.tensor_scalar(d_t[:], d_t[:], chsc[:, chn:chn + 1], 0.0,
                                Alu.add, Alu.add)
                nc.scalar.dma_start(out=y_ext[s, chn].rearrange("(c p) w -> p c w",
                                                                p=128),
                                    in_=cview(d_t)[:, :, :])

    nc.compile()
    return nc


def _get_program():
    if "nc" not in _CACHE:
        _CACHE["nc"] = _build()
    return _CACHE["nc"]


def kernel(x: np.ndarray) -> np.ndarray:
    from concourse.bass_utils import run_bass_kernel_spmd
    x = np.ascontiguousarray(np.asarray(x, dtype=np.float32))
    assert x.shape == (16, 3, H, W), x.shape
    nc = _get_program()
    consts = _host_consts()
    in_maps = [{"x": x[2 * i:2 * i + 2], **consts} for i in range(8)]
    res = run_bass_kernel_spmd(nc, in_maps, list(range(8)))
    out = np.concatenate([res.results[i]["y"] for i in range(8)], axis=0)
    return out.astype(np.float32)
